# revision 15
# baseline (speedup 1.0000x reference)
"""Trainium2 Bass kernel for nn_GroupFeatureBuilder (segment_reduce).

Shard M=4096 groups across 8 cores (512 each).  All gathers become dense
matmuls against a host-built multiplicity matrix C[m, n]:

  X = [attn_rr | h_hi | h_lo]  fp8e4, DoubleRow pair layout
  E = C@A (4 psum banks) + h_g hi/lo (1 bank) per 128-group chunk,
  one shared stationary ct slice per contraction pair.

  t1[m]  = <E[m], C[m]>          (fused DVE scalar_tensor_tensor, PSUM in)
  ao2[m] = <E[m], min(C,1)-1>    = t3 - esum
  a_in   = (t1 - t2) * inv_in    (t2, inv_in from host-gathered smalls)
  a_out  = ao2 * neginv_out
  h_g    = hi_g/16 + lo_g/256    (hi = fp8(h), lo = fp8((h-hi)*16))
  h_glob = ones^T hi/lo matmuls; a_obs/ex_dist/ex_clr/t2 from smalls.

Schedule: the first three E units + peH0 + hglob run j-major, paced by
the per-pair DMA arrivals of X; remaining units run unit-major so each
unit's PSUM drains (DVE) right behind the tensor engine.  Small
reductions/finals run on GpSimd to keep DVE on the big scans.
"""

import numpy as np
import ml_dtypes

import concourse.bass as bass
import concourse.bacc as bacc
import concourse.tile as tile
import concourse.mybir as mybir
from concourse.bass_utils import run_bass_kernel_spmd

BF16 = ml_dtypes.bfloat16
FP8 = ml_dtypes.float8_e4m3

N = 2048
D = 256
M = 4096
K = 16
NOBS = 64
NCORES = 8
MLOC = M // NCORES
MCH = MLOC // 128
KP = 8                 # DoubleRow k-pairs
XW = N + 2 * D         # 2560 = [A | h_hi | h_lo]
FOUT = 2 * D + 6
SMW = 84

f32 = mybir.dt.float32
bf16 = mybir.dt.bfloat16
fp8 = mybir.dt.float8e4
OP = mybir.AluOpType
AX = mybir.AxisListType
ACT = mybir.ActivationFunctionType
DR = mybir.MatmulPerfMode.DoubleRow

_NC_CACHE = {}


def _build_nc():
    nc = bacc.Bacc("TRN2", target_bir_lowering=False, debug=False,
                   num_devices=NCORES)

    a_d = nc.declare_dram_parameter("a_x", [128, KP * 2 * XW], fp8,
                                    isOutput=False)
    ct_d = nc.declare_dram_parameter("ct_x", [128, KP * 2 * MLOC], fp8,
                                     isOutput=False)
    cm_d = nc.declare_dram_parameter("cm_x", [128, MCH * N], bf16,
                                     isOutput=False)
    sm_d = nc.declare_dram_parameter("sm_x", [128, MCH * SMW], f32,
                                     isOutput=False)
    out_d = nc.declare_dram_parameter("out", [MLOC, FOUT], f32, isOutput=True)

    with tile.TileContext(nc) as tc:
        with (
            tc.tile_pool(name="res", bufs=1) as res,
            tc.tile_pool(name="om", bufs=3) as omp,
            tc.tile_pool(name="junk", bufs=2) as junkp,
            tc.tile_pool(name="outp", bufs=2) as outp,
            tc.tile_pool(name="stats", bufs=1) as statp,
            tc.tile_pool(name="psum_a", bufs=3, space="PSUM") as peA_pool,
            tc.tile_pool(name="psum_h", bufs=1, space="PSUM") as peH_pool,
            tc.tile_pool(name="psum_g", bufs=1, space="PSUM") as pg_pool,
        ):
            # ---- resident tiles; DMA emission order = arrival priority ----
            ct_all = res.tile([128, KP, 2, MLOC], fp8, tag="ct_all")
            a_all = res.tile([128, KP, 2, XW], fp8, tag="a_all")
            cm_all = res.tile([128, MCH, N], bf16, tag="cm_all")
            sm_all = res.tile([128, MCH, SMW], f32, tag="sm_all")

            nc.sync.dma_start(out=ct_all[:, 0:4, :, :],
                              in_=ct_d[:, 0:4 * 2 * MLOC])
            nc.sync.dma_start(out=a_all[:, 0, :, :], in_=a_d[:, 0:2 * XW])
            nc.sync.dma_start(out=ct_all[:, 4:8, :, :],
                              in_=ct_d[:, 4 * 2 * MLOC:])
            nc.sync.dma_start(out=cm_all[:, 0, :], in_=cm_d[:, 0:N])
            nc.sync.dma_start(out=cm_all[:, 1, :], in_=cm_d[:, N:2 * N])
            for j in range(1, KP):
                nc.sync.dma_start(out=a_all[:, j, :, :],
                                  in_=a_d[:, j * 2 * XW:(j + 1) * 2 * XW])
            nc.sync.dma_start(out=cm_all[:, 2, :], in_=cm_d[:, 2 * N:3 * N])
            nc.sync.dma_start(out=cm_all[:, 3, :], in_=cm_d[:, 3 * N:4 * N])
            nc.sync.dma_start(out=sm_all[:], in_=sm_d[:, :])

            ones_p = res.tile([128, 2, 1], fp8, tag="ones_p")
            nc.vector.memset(ones_p[:], 1.0)
            ones_row = res.tile([1, 128], bf16, tag="ones_row")
            nc.vector.memset(ones_row[:], 1.0)
            hglob_b = res.tile([128, D], f32, tag="hglob_b")

            peA = {}
            peH = {}
            out_t = {}
            oms = {}
            # per-m stats tile: 0 t1h0 | 1 t1h1 | 2 aoh0 | 3 aoh1 | 4 t2 |
            #                   5 exd | 6 aob | 7 t1s
            st = {}

            def alloc_m(m):
                out_t[m] = outp.tile([128, FOUT], f32, tag="out",
                                     name=f"out{m}")
                st[m] = statp.tile([128, 8], f32, tag=f"st{m}",
                                   name=f"st{m}")

            def emit_A(m, h, j):
                for s in range(2):
                    c0 = h * 1024 + s * 512
                    nc.tensor.matmul(
                        peA[(m, h)][:, s * 512:(s + 1) * 512],
                        ct_all[:, j, :, m * 128:(m + 1) * 128],
                        a_all[:, j, :, c0:c0 + 512],
                        start=(j == 0), stop=(j == KP - 1), perf_mode=DR)

            def emit_H(m, j):
                nc.tensor.matmul(peH[m][:], ct_all[:, j, :,
                                                   m * 128:(m + 1) * 128],
                                 a_all[:, j, :, N:XW],
                                 start=(j == 0), stop=(j == KP - 1),
                                 perf_mode=DR)

            def drain_om(m):
                oms[m] = omp.tile([128, N], bf16, tag="om", name=f"om{m}")
                nc.vector.tensor_scalar(out=oms[m][:], in0=cm_all[:, m, :],
                                        scalar1=1.0, scalar2=-1.0,
                                        op0=OP.min, op1=OP.add)

            def drain_A(m, h):
                c0 = h * 1024
                jk = junkp.tile([128, 1024], bf16, tag="jk")
                nc.vector.scalar_tensor_tensor(
                    out=jk[:], in0=peA[(m, h)][:], scalar=1.0,
                    in1=cm_all[:, m, c0:c0 + 1024],
                    op0=OP.mult, op1=OP.mult, accum_out=st[m][:, h:h + 1])
                jk2 = junkp.tile([128, 1024], bf16, tag="jk")
                nc.vector.scalar_tensor_tensor(
                    out=jk2[:], in0=peA[(m, h)][:], scalar=1.0,
                    in1=oms[m][:, c0:c0 + 1024],
                    op0=OP.mult, op1=OP.mult,
                    accum_out=st[m][:, 2 + h:3 + h])

            def drain_m(m):
                ot = out_t[m]
                s = st[m]
                # h_g = hi_g/16 + lo_g/256
                hgs = statp.tile([128, D], f32, tag=f"hgs{m}",
                                 name=f"hgs{m}")
                nc.scalar.activation(hgs[:], peH[m][:, 0:D], ACT.Copy,
                                     scale=1.0 / K)
                nc.vector.scalar_tensor_tensor(
                    out=ot[:, 0:D], in0=peH[m][:, D:2 * D],
                    scalar=1.0 / (K * 16.0), in1=hgs[:],
                    op0=OP.mult, op1=OP.add)
                nc.scalar.activation(ot[:, D:2 * D], hglob_b[:], ACT.Copy)
                # smalls on gpsimd (plain tensor_scalar accumulations)
                jks = statp.tile([128, 16], f32, tag=f"jks{m}",
                                 name=f"jks{m}")
                nc.vector.tensor_scalar(out=jks[:], in0=sm_all[:, m, 32:48],
                                        scalar1=1.0, scalar2=0.0,
                                        op0=OP.mult, op1=OP.add,
                                        accum_out=s[:, 4:5])
                jk2 = statp.tile([128, 16], f32, tag=f"jk2_{m}",
                                 name=f"jk2_{m}")
                nc.vector.tensor_scalar(out=jk2[:], in0=sm_all[:, m, 0:16],
                                        scalar1=1.0 / K, scalar2=0.0,
                                        op0=OP.mult, op1=OP.add,
                                        accum_out=ot[:, 516:517])
                jk3 = statp.tile([128, 16], f32, tag=f"jk3_{m}",
                                 name=f"jk3_{m}")
                nc.vector.tensor_scalar(out=jk3[:], in0=sm_all[:, m, 64:80],
                                        scalar1=1.0 / (K * NOBS),
                                        scalar2=0.0, op0=OP.mult, op1=OP.add,
                                        accum_out=ot[:, 515:516])
                nc.vector.tensor_reduce(ot[:, 517:518], sm_all[:, m, 16:32],
                                        AX.X, OP.min)
                # a_in = ((t1h0 + t1h1) - t2) * inv_in
                nc.vector.tensor_sub(s[:, 7:8], s[:, 0:1], s[:, 4:5])
                nc.vector.scalar_tensor_tensor(
                    out=ot[:, 513:514], in0=s[:, 7:8], scalar=s[:, 1:2],
                    in1=sm_all[:, m, 81:82], op0=OP.add, op1=OP.mult)
                # a_out = (aoh0 + aoh1) * neginv_out
                nc.vector.scalar_tensor_tensor(
                    out=ot[:, 514:515], in0=s[:, 2:3], scalar=s[:, 3:4],
                    in1=sm_all[:, m, 80:81], op0=OP.add, op1=OP.mult)
                nc.vector.memset(ot[:, 512:513], float(K) / 3.0)
                nc.sync.dma_start(out=out_d[m * 128:(m + 1) * 128, :],
                                  in_=ot[:])

            # ================= window: j-major, paced by a_j DMAs =========
            alloc_m(0)
            alloc_m(1)
            for (m, h) in [(0, 0), (0, 1), (1, 0)]:
                peA[(m, h)] = peA_pool.tile([128, 1024], f32, tag="peA",
                                            name=f"peA{m}_{h}")
            peH[0] = peH_pool.tile([128, 512], f32, tag="peH", name="peH0")
            pg = pg_pool.tile([1, 512], f32, tag="pg", name="pg")
            drain_om(0)
            drain_om(1)
            for h in range(2):
                for j in range(KP):
                    emit_A(0, h, j)
            for j in range(KP):
                emit_H(0, j)
                for i in range(2):
                    nc.tensor.matmul(pg[:], ones_p[:, i, 0:1],
                                     a_all[:, j, i, N:XW],
                                     start=(j == 0 and i == 0),
                                     stop=(j == KP - 1 and i == 1))
            for j in range(KP):
                emit_A(1, 0, j)

            # hglob combine + broadcast
            hgfs = statp.tile([1, D], f32, tag="hgfs")
            nc.scalar.activation(hgfs[:], pg[:, 0:D], ACT.Copy)
            hgf = statp.tile([1, D], f32, tag="hgf")
            nc.vector.scalar_tensor_tensor(
                out=hgf[:], in0=pg[:, D:2 * D], scalar=1.0 / 16.0,
                in1=hgfs[:], op0=OP.mult, op1=OP.add)
            hgrow = statp.tile([1, D], bf16, tag="hgrow")
            nc.scalar.activation(hgrow[:], hgf[:], ACT.Copy, scale=1.0 / N)
            pgb = pg_pool.tile([128, D], f32, tag="pg", name="pgb")
            nc.tensor.matmul(pgb[:], ones_row[:1], hgrow[:1])
            nc.scalar.activation(hglob_b[:], pgb[:], ACT.Copy)

            # window drains
            drain_A(0, 0)
            drain_A(0, 1)
            drain_m(0)
            drain_A(1, 0)

            # ================= tail: unit-major, drains chase ==============
            peA[(1, 1)] = peA_pool.tile([128, 1024], f32, tag="peA",
                                        name="peA1_1")
            for j in range(KP):
                emit_A(1, 1, j)
            peH[1] = peH_pool.tile([128, 512], f32, tag="peH", name="peH1")
            for j in range(KP):
                emit_H(1, j)
            drain_A(1, 1)
            drain_m(1)

            for m in (2, 3):
                alloc_m(m)
                drain_om(m)
                for h in range(2):
                    peA[(m, h)] = peA_pool.tile([128, 1024], f32, tag="peA",
                                                name=f"peA{m}_{h}")
                    for j in range(KP):
                        emit_A(m, h, j)
                    drain_A(m, h)
                peH[m] = peH_pool.tile([128, 512], f32, tag="peH",
                                       name=f"peH{m}")
                for j in range(KP):
                    emit_H(m, j)
                drain_m(m)
    nc.compile()
    return nc


def _get_nc():
    if "nc" not in _NC_CACHE:
        _NC_CACHE["nc"] = _build_nc()
    return _NC_CACHE["nc"]


def _pair_layout(x):
    """[2048, w] -> [128, 8*2*w]: row p holds [X[2j*128+p,:] | X[(2j+1)*128+p,:]]."""
    w = x.shape[1]
    return np.ascontiguousarray(
        x.reshape(KP, 2, 128, w).transpose(2, 0, 1, 3).reshape(128,
                                                               KP * 2 * w))


def kernel(h, attn_rr, attn_ro, dist_to_goal, clearance, groups):
    h = np.asarray(h, dtype=np.float32)
    attn_rr = np.asarray(attn_rr, dtype=np.float32)
    attn_ro = np.asarray(attn_ro, dtype=np.float32)
    dist_to_goal = np.asarray(dist_to_goal, dtype=np.float32)
    clearance = np.asarray(clearance, dtype=np.float32)
    groups = np.asarray(groups)

    h_hi = h.astype(FP8)
    h_lo = ((h - h_hi.astype(np.float32)) * 16.0).astype(FP8)
    x_full = np.concatenate([attn_rr.astype(FP8), h_hi, h_lo], axis=1)
    a_x = _pair_layout(x_full)
    robs = attn_ro.sum(axis=1, dtype=np.float32)
    diag = np.ascontiguousarray(np.diagonal(attn_rr)).astype(np.float32)

    in_maps = []
    for s in range(NCORES):
        gs = groups[s * MLOC:(s + 1) * MLOC]
        C = np.zeros((MLOC, N), dtype=np.float32)
        np.add.at(C, (np.arange(MLOC)[:, None], gs), 1.0)

        sumcc = (C * C).sum(axis=1)
        nuniq = (C > 0).sum(axis=1).astype(np.float32)
        sm = np.zeros((MLOC, SMW), dtype=np.float32)
        sm[:, 0:16] = dist_to_goal[gs]
        sm[:, 16:32] = clearance[gs]
        sm[:, 32:48] = diag[gs] * C[np.arange(MLOC)[:, None], gs]
        sm[:, 64:80] = robs[gs]
        sm[:, 80] = -1.0 / (K * (N - nuniq))
        sm[:, 81] = 1.0 / np.maximum(K * K - sumcc, 1.0)

        in_maps.append({
            "a_x": a_x,
            "ct_x": _pair_layout(C.T.astype(FP8)),
            "cm_x": np.ascontiguousarray(
                C.astype(BF16).reshape(MCH, 128, N).transpose(1, 0, 2)
                .reshape(128, MCH * N)),
            "sm_x": np.ascontiguousarray(
                sm.reshape(MCH, 128, SMW).transpose(1, 0, 2)
                .reshape(128, MCH * SMW)),
        })

    nc = _get_nc()
    _NC_CACHE["last_in_maps"] = in_maps
    res = run_bass_kernel_spmd(nc, in_maps, list(range(NCORES)))
    return np.concatenate([res.results[s]["out"] for s in range(NCORES)],
                          axis=0)
